# revision 1
# baseline (speedup 1.0000x reference)
"""Trainium2 kernel for a video-diffusion BasicTransformerBlock.

Strategy: all heavy matmuls (QKV/out projections, GEGLU FFN) run on the 8
NeuronCores as Bass/Tile programs, sharded data-parallel over the 8192
token rows (batch-frame x spatial). LayerNorm stats, softmax, gelu gating
and residual adds are cheap vector ops done host-side between launches.

Five small Bass programs (compiled once, cached in-process):
  G1  : [1280 x 1024] x [1280 x 3840]  -> fused q|k|v projection
  G2  : [1280 x 1024] x [1280 x 1280]  -> single D x D projection
  GE  : [768  x  512] x [768  x 2560]  -> encoder k|v projection
  GF1 : [1280 x 1024] x [1280 x 10240] -> GEGLU up-projection
  GF2 : [5120 x 1024] x [5120 x 1280]  -> FFN down-projection

Each program computes outT[M, Ntok] = W[K, M].T @ xT[K, Ntok] on every
core (xT differs per core; W identical). fp32 data, fp32r matmuls.
"""
import sys
import numpy as np

sys.path.insert(0, "/opt/trn_rl_repo")

import concourse.bass as bass
import concourse.tile as tile
from concourse import mybir
from concourse.bass_utils import run_bass_kernel_spmd

HEADS = 20
DH = 64
SCALE = DH ** -0.5
D = 1280
DC = 768
BF, N, F = 32, 256, 16
B = BF // F
NCORES = 8
ROWS = (BF * N) // NCORES  # 1024 rows per core

_PROGRAMS = {}


def _build_matmul_program(K, M, Ntok):
    """outT[M, Ntok] = W[K, M].T @ xT[K, Ntok], fp32r matmuls, fp32 io.

    Raw bass with explicit semaphores (one wait per instruction):
    gpsimd streams x (per 512-token half) and w (double-buffered per
    128-col tile of M), tensor runs the K-accumulation, vector copies
    PSUM->SBUF, sync writes results out. Steady state keeps the PE busy.
    """
    nc = bass.Bass()
    x_in = nc.declare_dram_parameter("x", [K, Ntok], mybir.dt.float32, isOutput=False)
    w_in = nc.declare_dram_parameter("w", [K, M], mybir.dt.float32, isOutput=False)
    y_out = nc.declare_dram_parameter("y", [M, Ntok], mybir.dt.float32, isOutput=True)
    KC = K // 128
    MC = M // 128
    NT = Ntok // 512
    f32r = mybir.dt.float32r
    f32 = mybir.dt.float32
    with (
        nc.semaphore("dma_in") as dma_in,
        nc.semaphore("mm_sem") as mm_sem,
        nc.semaphore("cp_sem") as cp_sem,
        nc.semaphore("out_sem") as out_sem,
        nc.sbuf_tensor("xt", [128, KC, 512], f32r) as xt,
        nc.sbuf_tensor("wt", [128, 2, KC, 128], f32r) as wt,
        nc.sbuf_tensor("obuf", [128, 2, 512], f32) as obuf,
        nc.psum_tensor("acc", [128, 2, 512], f32) as acc,
    ):
        with nc.Block() as block:

            @block.gpsimd
            def _(gpsimd):
                nloads = 0
                for nt in range(NT):
                    if nt >= 1:
                        gpsimd.wait_ge(cp_sem, nt * MC)
                    gpsimd.dma_start(
                        out=xt[:],
                        in_=x_in[:, nt * 512:(nt + 1) * 512].rearrange(
                            "(kc p) n -> p kc n", p=128),
                    ).then_inc(dma_in, 16)
                    nloads += 1
                    for m in range(MC):
                        g = nt * MC + m
                        if g >= 2:
                            gpsimd.wait_ge(cp_sem, g - 1)
                        gpsimd.dma_start(
                            out=wt[:, g % 2],
                            in_=w_in[:, m * 128:(m + 1) * 128].rearrange(
                                "(kc p) m -> p kc m", p=128),
                        ).then_inc(dma_in, 16)
                        nloads += 1

            @block.tensor
            def _(tensor):
                for nt in range(NT):
                    for m in range(MC):
                        g = nt * MC + m
                        if g >= 2:
                            tensor.wait_ge(cp_sem, g - 1)
                        # x load for this nt plus w loads up to g inclusive
                        tensor.wait_ge(dma_in, 16 * (nt + 1 + g + 1))
                        for kc in range(KC):
                            ins = tensor.matmul(
                                acc[:, g % 2],
                                wt[:, g % 2, kc],
                                xt[:, kc],
                                start=(kc == 0),
                                stop=(kc == KC - 1),
                            )
                        ins.then_inc(mm_sem, 1)

            @block.vector
            def _(vector):
                for g in range(NT * MC):
                    vector.wait_ge(mm_sem, g + 1)
                    if g >= 2:
                        vector.wait_ge(out_sem, 16 * (g - 1))
                    vector.tensor_copy(
                        out=obuf[:, g % 2], in_=acc[:, g % 2]
                    ).then_inc(cp_sem, 1)

            @block.sync
            def _(sync):
                for nt in range(NT):
                    for m in range(MC):
                        g = nt * MC + m
                        sync.wait_ge(cp_sem, g + 1)
                        sync.dma_start(
                            out=y_out[m * 128:(m + 1) * 128,
                                      nt * 512:(nt + 1) * 512],
                            in_=obuf[:, g % 2],
                        ).then_inc(out_sem, 16)
    return nc


def _get_program(K, M, Ntok):
    key = (K, M, Ntok)
    if key not in _PROGRAMS:
        _PROGRAMS[key] = _build_matmul_program(K, M, Ntok)
    return _PROGRAMS[key]


def _device_matmul(xT_shards, W):
    """xT_shards: list of NCORES arrays [K, Ntok]; W: [K, M] fp32.
    Returns list of NCORES arrays [M, Ntok]."""
    K, Ntok = xT_shards[0].shape
    M = W.shape[1]
    nc = _get_program(K, M, Ntok)
    W = np.ascontiguousarray(W, dtype=np.float32)
    in_maps = [{"x": np.ascontiguousarray(x, dtype=np.float32), "w": W}
               for x in xT_shards]
    res = run_bass_kernel_spmd(nc, in_maps, list(range(NCORES)))
    return [r["y"] for r in res.results]


def _mm(x, W):
    """x: [rows, K] fp32 (rows == 8192 or padded multiple of NCORES*512...).
    Computes x @ W on the 8 cores, sharding rows. Returns [rows, M]."""
    rows = x.shape[0]
    per = rows // NCORES
    xT_shards = [x[i * per:(i + 1) * per].T for i in range(NCORES)]
    outs = _device_matmul(xT_shards, W)
    return np.concatenate([o.T for o in outs], axis=0)


def _ln(x, w, b):
    x = x.astype(np.float32)
    m = x.mean(-1, keepdims=True)
    v = ((x - m) ** 2).mean(-1, keepdims=True)
    return (x - m) / np.sqrt(v + 1e-5) * w + b


def _softmax(s):
    s = s - s.max(-1, keepdims=True)
    e = np.exp(s)
    return e / e.sum(-1, keepdims=True)


def _heads(t, nrows):
    # [rows, D] -> [batch, H, n, dh] given rows = batch*n
    bsz = t.shape[0] // nrows
    return t.reshape(bsz, nrows, HEADS, DH).transpose(0, 2, 1, 3)


def _unheads(t):
    b, h, n, dh = t.shape
    return t.transpose(0, 2, 1, 3).reshape(b * n, h * dh)


def kernel(hidden_states, encoder_hidden_states, norm1_w, norm1_b,
           a1_q, a1_k, a1_v, a1_ow, a1_ob,
           norm2_w, norm2_b, a2_q, a2_k, a2_v, a2_ow, a2_ob,
           norm3_w, norm3_b, ff1_w, ff1_b, ff2_w, ff2_b,
           normt_w, normt_b, at_q, at_k, at_v, at_ow, at_ob,
           pb1_w, pb1_b, pb2_w, pb2_b, pb3_w, pb3_b, video_length):
    f = int(video_length)
    x = np.asarray(hidden_states, dtype=np.float32)
    enc = np.asarray(encoder_hidden_states, dtype=np.float32)
    bf, n, d = x.shape
    b = bf // f
    rows = bf * n
    xr = x.reshape(rows, d)

    # ---- attn1: sparse-causal self-attention ----
    nx = _ln(xr, norm1_w, norm1_b)
    qkv_w = np.concatenate([a1_q, a1_k, a1_v], axis=1)  # [D, 3D]
    qkv = _mm(nx, qkv_w)
    q, k, v = qkv[:, :d], qkv[:, d:2 * d], qkv[:, 2 * d:]

    qh = _heads(q, n) * SCALE                      # [bf, H, n, dh]
    kh = _heads(k, n).reshape(b, f, HEADS, n, DH)
    vh = _heads(v, n).reshape(b, f, HEADS, n, DH)
    former = np.concatenate([[0], np.arange(f - 1)]).astype(np.int64)
    # KV = concat([frame0, former frame]) along keys
    kcat = np.concatenate(
        [np.broadcast_to(kh[:, :1], kh.shape), kh[:, former]], axis=3)
    vcat = np.concatenate(
        [np.broadcast_to(vh[:, :1], vh.shape), vh[:, former]], axis=3)
    kcat = kcat.reshape(bf, HEADS, 2 * n, DH)
    vcat = vcat.reshape(bf, HEADS, 2 * n, DH)
    sim = np.einsum('bhid,bhjd->bhij', qh, kcat, optimize=True)
    attn = _softmax(sim)
    o = np.einsum('bhij,bhjd->bhid', attn, vcat, optimize=True)
    o = _unheads(o)                                # [rows, D]
    xr = _mm(o, a1_ow) + a1_ob + xr

    # ---- attn2: cross-attention to encoder states ----
    nx = _ln(xr, norm2_w, norm2_b)
    q2 = _mm(nx, a2_q)
    # encoder kv: pad 77 -> 128 rows per frame, shard over cores
    L = enc.shape[1]
    encp = np.zeros((bf, 128, DC), dtype=np.float32)
    encp[:, :L] = enc
    encp = encp.reshape(bf * 128, DC)
    kv2_w = np.concatenate([a2_k, a2_v], axis=1)   # [DC, 2D]
    kv2 = _mm(encp, kv2_w).reshape(bf, 128, 2 * d)[:, :L]
    k2, v2 = kv2[:, :, :d], kv2[:, :, d:]
    q2h = _heads(q2, n) * SCALE
    k2h = _heads(k2.reshape(bf * L, d), L)
    v2h = _heads(v2.reshape(bf * L, d), L)
    sim2 = np.einsum('bhid,bhjd->bhij', q2h, k2h, optimize=True)
    o2 = np.einsum('bhij,bhjd->bhid', _softmax(sim2), v2h, optimize=True)
    xr = _mm(_unheads(o2), a2_ow) + a2_ob + xr

    # ---- GEGLU feed-forward ----
    nx = _ln(xr, norm3_w, norm3_b)
    hg = _mm(nx, ff1_w) + ff1_b
    hh, gate = hg[:, :4 * d], hg[:, 4 * d:]
    # exact gelu
    from math import sqrt
    try:
        from scipy.special import erf
        g = 0.5 * gate * (1.0 + erf(gate / sqrt(2.0)))
    except ImportError:
        from numpy import vectorize
        import math
        g = 0.5 * gate * (1.0 + np.vectorize(math.erf)(gate / sqrt(2.0)))
    xr = _mm(hh * g, ff2_w) + ff2_b + xr

    # ---- temporal attention with relative position bias ----
    rel = np.arange(-f + 1, f, dtype=np.float32)[:, None]
    def silu(t):
        return t / (1.0 + np.exp(-t))
    hb = silu(rel @ pb1_w + pb1_b)
    hb = silu(hb @ pb2_w + pb2_b)
    tab = hb @ pb3_w + pb3_b                        # [2f-1, H]
    idx = np.arange(f)[:, None] - np.arange(f)[None, :] + (f - 1)
    bias = tab[idx].transpose(2, 0, 1)[None]        # [1, H, f, f]

    xt = xr.reshape(b, f, n, d).transpose(0, 2, 1, 3).reshape(b * n, f, d)
    xtr = xt.reshape(b * n * f, d)
    nx = _ln(xtr, normt_w, normt_b)
    qkvt_w = np.concatenate([at_q, at_k, at_v], axis=1)
    qkvt = _mm(nx, qkvt_w)
    qt, kt, vt = qkvt[:, :d], qkvt[:, d:2 * d], qkvt[:, 2 * d:]
    qth = _heads(qt, f) * SCALE                     # [b*n, H, f, dh]
    kth = _heads(kt, f)
    vth = _heads(vt, f)
    simt = np.einsum('bhid,bhjd->bhij', qth, kth, optimize=True) + bias
    ot = np.einsum('bhij,bhjd->bhid', _softmax(simt), vth, optimize=True)
    xtr = _mm(_unheads(ot), at_ow) + at_ob + xtr

    out = xtr.reshape(b, n, f, d).transpose(0, 2, 1, 3).reshape(bf, n, d)
    return out.astype(np.float32)

